# revision 5
# baseline (speedup 1.0000x reference)
"""Trainium2 Bass kernel for BeliveMapsNMS (7x7 NMS + per-map top-100 peaks).

Strategy
--------
Input is belive_map [4, 25, 1024, 1024] f32 (400 MiB). The output is tiny
(top-100 local maxima per (b, s) map), so the kernel is a streaming
reduction bounded by HBM read bandwidth (memory regime).

Key algebraic fact: a pixel that is the max of its 7x7 window is always the
max of its aligned 4x4 cell (cell diameter 3 <= window radius 3). So a 16:1
cell-max reduction preserves every NMS peak. On each core:

  per map [1024,1024]:
    - load as one [128, 8192] SBUF tile (partition p holds pixel rows 8p..8p+7)
    - 7 tensor_tensor max ops reduce to 4x4 cell maxima cm [128, 512]
      (partition p holds cell rows 2p, 2p+1; 256 cell cols each)
    - 2 rounds of (max8 -> max_index -> match_replace) extract the top-16
      cells per partition (values + in-row indices)

Each (b,s) map contributes >= its top-16-cells-per-8-row-stripe; the global
top-100 peaks of a map are spread across 128 stripes (expected <1 per
stripe), so top-16 per stripe is a vast superset of the needed candidates.

Sharding: 100 maps flattened, padded to 104 = 8 cores x 13 maps. Pure data
parallel, no collectives.

Host post-pass (O(candidates), ~0.05% of input): for ~2048 candidate cells
per map, read the 4x4 patch from the (host-resident) input, keep pixels
achieving the cell max, check the exact 7x7 window, then stable top-100 by
(value desc, flat index asc) to match lax.top_k tie-breaking.
"""

import numpy as np

B, S, H, W = 4, 25, 1024, 1024
NMAPS = B * S            # 100
NCORES = 8
MAPS_PER_CORE = 13       # 8 * 13 = 104 (4 zero-padded maps)
K = 100
MIN_DISTANCE = 3
THR = np.float32(2.0 / (H * W))
NEG = -3.0e38

_NC = None


def _build():
    import concourse.tile as tile
    from concourse import bacc, mybir

    f32 = mybir.dt.float32
    u32 = mybir.dt.uint32

    nc = bacc.Bacc("TRN2", target_bir_lowering=False, debug=False)
    x = nc.dram_tensor("x", [MAPS_PER_CORE, H, W], f32, kind="ExternalInput")
    vals = nc.dram_tensor(
        "vals", [MAPS_PER_CORE, 128, 16], f32, kind="ExternalOutput"
    )
    idxs = nc.dram_tensor(
        "idxs", [MAPS_PER_CORE, 128, 16], u32, kind="ExternalOutput"
    )

    with tile.TileContext(nc) as tc:
        with (
            tc.tile_pool(name="inp", bufs=3) as inp,
            tc.tile_pool(name="mid", bufs=2) as mid,
            tc.tile_pool(name="outp", bufs=3) as outp,
        ):
            for m in range(MAPS_PER_CORE):
                # ---- load map: partition p <- pixel rows 8p..8p+7 (32 KiB
                # contiguous per partition); 4 DMAs to spread across queues.
                t = inp.tile([128, 8192], f32)
                src = x[m].rearrange("(p k) j -> p k j", k=8)  # [128, 8, 1024]
                for g in range(4):
                    nc.sync.dma_start(
                        out=t[:, g * 2048 : (g + 1) * 2048],
                        in_=src[:, 2 * g : 2 * g + 2, :],
                    )

                # ---- vertical 4:1 within cells: rows (4r..4r+3) -> cell row
                # free-dim chunk k holds pixel row 8p+k. Pair (0,1),(2,3),...
                a = mid.tile([128, 4096], f32)
                for u in range(4):
                    nc.vector.tensor_max(
                        a[:, u * 1024 : (u + 1) * 1024],
                        t[:, (2 * u) * 1024 : (2 * u + 1) * 1024],
                        t[:, (2 * u + 1) * 1024 : (2 * u + 2) * 1024],
                    )
                # a chunks: [r01, r23, r45, r67]; combine to v = [cellrow 2p | 2p+1]
                v = mid.tile([128, 2048], f32)
                a4 = a[:].rearrange("p (u j) -> p u j", j=1024)  # [128,4,1024]
                nc.vector.tensor_max(v[:], a4[:, 0::2, :], a4[:, 1::2, :])

                # ---- horizontal 4:1: cols (4c..4c+3) -> cell col
                h1 = mid.tile([128, 1024], f32)
                v2 = v[:].rearrange("p (j two) -> p j two", two=2)  # [128,1024,2]
                nc.vector.tensor_max(h1[:], v2[:, :, 0], v2[:, :, 1])
                cm = mid.tile([128, 512], f32)
                h2 = h1[:].rearrange("p (j two) -> p j two", two=2)
                nc.vector.tensor_max(cm[:], h2[:, :, 0], h2[:, :, 1])

                # ---- top-16 cells per partition (2 rounds of max8);
                # separate tiles per round so each out-DMA has one writer.
                mv1 = outp.tile([128, 8], f32)
                mi1 = outp.tile([128, 8], u32)
                mv2 = outp.tile([128, 8], f32)
                mi2 = outp.tile([128, 8], u32)
                nc.vector.max(mv1[:], cm[:])
                nc.vector.max_index(mi1[:], mv1[:], cm[:])
                cm2 = mid.tile([128, 512], f32)
                nc.vector.match_replace(cm2[:], mv1[:], cm[:], NEG)
                nc.vector.max(mv2[:], cm2[:])
                nc.vector.max_index(mi2[:], mv2[:], cm2[:])

                nc.gpsimd.dma_start(out=vals[m, :, 0:8], in_=mv1[:])
                nc.gpsimd.dma_start(out=vals[m, :, 8:16], in_=mv2[:])
                nc.gpsimd.dma_start(out=idxs[m, :, 0:8], in_=mi1[:])
                nc.gpsimd.dma_start(out=idxs[m, :, 8:16], in_=mi2[:])
    nc.compile()
    return nc


def _get_nc():
    global _NC
    if _NC is None:
        _NC = _build()
    return _NC


def run_device(xpad: np.ndarray, trace: bool = False):
    """Run the Bass kernel on 8 cores. xpad: [104, H, W] f32.

    Returns (vals [104,128,16] f32, idxs [104,128,16] uint32, results obj).
    """
    from concourse.bass_utils import run_bass_kernel_spmd

    nc = _get_nc()
    in_maps = [
        {"x": xpad[i * MAPS_PER_CORE : (i + 1) * MAPS_PER_CORE]}
        for i in range(NCORES)
    ]
    res = run_bass_kernel_spmd(nc, in_maps, list(range(NCORES)), trace=trace)
    vals = np.concatenate([r["vals"] for r in res.results], axis=0)
    idxs = np.concatenate([r["idxs"] for r in res.results], axis=0)
    return vals, idxs, res


def postprocess(x: np.ndarray, vals: np.ndarray, idxs: np.ndarray):
    """Exact NMS + stable top-100 from candidate cells.

    x: [NMAPS, H, W] f32 full input; vals/idxs: [NMAPS, 128, 16].
    """
    ncand = 128 * 16
    m_id = np.repeat(np.arange(NMAPS, dtype=np.int64), ncand)
    p = np.tile(np.repeat(np.arange(128, dtype=np.int64), 16), NMAPS)
    f = idxs[:NMAPS].reshape(-1).astype(np.int64)
    v_cand = vals[:NMAPS].reshape(-1)

    # Per-map trim to top-400 cells by value (candidates are cell maxima;
    # top-100 peaks lie within the top ~110 cells).
    keep = np.zeros(v_cand.size, dtype=bool)
    vc2 = v_cand.reshape(NMAPS, ncand)
    kth = np.partition(vc2, ncand - 400, axis=1)[:, ncand - 400]
    keep = (vc2 >= np.maximum(kth, THR)[:, None]).reshape(-1)
    m_id, p, f, v_cand = m_id[keep], p[keep], f[keep], v_cand[keep]

    # Decode cell coordinates: partition p holds cell rows 2p (f<256) and
    # 2p+1 (f>=256); cell col = f % 256.
    cr = 2 * p + (f >= 256)
    cc = f % 256

    # Pixels achieving the cell max (includes exact ties).
    d4 = np.arange(4, dtype=np.int64)
    py = 4 * cr[:, None, None] + d4[None, :, None]
    px = 4 * cc[:, None, None] + d4[None, None, :]
    patch = x[m_id[:, None, None], py, px]  # [N,4,4]
    pmax = patch.max(axis=(1, 2))
    sel = patch == pmax[:, None, None]
    ci, iy, ix = np.nonzero(sel)
    my = m_id[ci]
    yy = 4 * cr[ci] + iy
    xx = 4 * cc[ci] + ix
    vv = x[my, yy, xx]

    good = vv > THR
    my, yy, xx, vv = my[good], yy[good], xx[good], vv[good]

    # Exact 7x7 window max (border-clipped).
    wm = np.full(vv.shape, -np.inf, dtype=np.float32)
    for dy in range(-MIN_DISTANCE, MIN_DISTANCE + 1):
        y2 = yy + dy
        oky = (y2 >= 0) & (y2 < H)
        y2c = np.clip(y2, 0, H - 1)
        for dx in range(-MIN_DISTANCE, MIN_DISTANCE + 1):
            x2 = xx + dx
            ok = oky & (x2 >= 0) & (x2 < W)
            nb = x[my, y2c, np.clip(x2, 0, W - 1)]
            np.maximum(wm, np.where(ok, nb, -np.inf), out=wm)
    is_peak = vv == wm
    my, yy, xx, vv = my[is_peak], yy[is_peak], xx[is_peak], vv[is_peak]

    flat = yy * W + xx
    skeletons = np.zeros((NMAPS, K, 3), dtype=np.int32)
    scores = np.full((NMAPS, K), -np.inf, dtype=np.float32)
    order_all = np.argsort(my, kind="stable")
    my, flat, vv = my[order_all], flat[order_all], vv[order_all]
    bounds = np.searchsorted(my, np.arange(NMAPS + 1))
    for m in range(NMAPS):
        lo, hi = bounds[m], bounds[m + 1]
        fl, vm = flat[lo:hi], vv[lo:hi]
        if fl.size < K:
            raise RuntimeError(
                f"map {m}: only {fl.size} candidate peaks (< {K})"
            )
        o = np.lexsort((fl, -vm))[:K]
        fk, vk = fl[o], vm[o]
        seg = m % S
        skeletons[m, :, 0] = seg
        skeletons[m, :, 1] = (fk % W).astype(np.int32)
        skeletons[m, :, 2] = (fk // W).astype(np.int32)
        scores[m] = vk
    return (
        skeletons.reshape(B, S, K, 3),
        scores.reshape(B, S, K),
    )


def kernel(belive_map):
    x = np.ascontiguousarray(np.asarray(belive_map, dtype=np.float32)).reshape(
        NMAPS, H, W
    )
    xpad = np.concatenate(
        [x, np.zeros((NCORES * MAPS_PER_CORE - NMAPS, H, W), dtype=np.float32)],
        axis=0,
    )
    vals, idxs, _ = run_device(xpad, trace=False)
    return postprocess(x, vals, idxs)


# revision 7
# speedup vs baseline: 1.1893x; 1.1893x over previous
"""Trainium2 Bass kernel for BeliveMapsNMS (7x7 NMS + per-map top-100 peaks).

Strategy
--------
Input is belive_map [4, 25, 1024, 1024] f32 (400 MiB). The output is tiny
(top-100 local maxima per (b, s) map), so the kernel is a streaming
reduction bounded by HBM read bandwidth (memory regime).

Key algebraic fact: a pixel that is the max of its 7x7 window is always the
max of its aligned 4x4 cell (cell diameter 3 <= window radius 3). So a 16:1
cell-max reduction preserves every NMS peak. On each core:

  per map [1024,1024]:
    - load as one [128, 8192] SBUF tile (partition p holds pixel rows 8p..8p+7)
    - 7 tensor_tensor max ops reduce to 4x4 cell maxima cm [128, 512]
      (partition p holds cell rows 2p, 2p+1; 256 cell cols each)
    - 2 rounds of (max8 -> max_index -> match_replace) extract the top-16
      cells per partition (values + in-row indices)

Each (b,s) map contributes >= its top-16-cells-per-8-row-stripe; the global
top-100 peaks of a map are spread across 128 stripes (expected <1 per
stripe), so top-16 per stripe is a vast superset of the needed candidates.

Sharding: 100 maps flattened, padded to 104 = 8 cores x 13 maps. Pure data
parallel, no collectives.

Host post-pass (O(candidates), ~0.05% of input): for ~2048 candidate cells
per map, read the 4x4 patch from the (host-resident) input, keep pixels
achieving the cell max, check the exact 7x7 window, then stable top-100 by
(value desc, flat index asc) to match lax.top_k tie-breaking.
"""

import numpy as np

B, S, H, W = 4, 25, 1024, 1024
NMAPS = B * S            # 100
NCORES = 8
MAPS_PER_CORE = 13       # 8 * 13 = 104 (4 zero-padded maps)
K = 100
MIN_DISTANCE = 3
THR = np.float32(2.0 / (H * W))
NEG = -3.0e38

_NC = None

# Tuning knobs (variant-bench'd on HW):
#   ROUNDS: max8 extraction rounds (1 round = top-8/partition, 2 = top-16)
#   H_ENGINE: engine for the two horizontal reduce levels ("vector"/"gpsimd")
#   V2_ENGINE: engine for the 2nd vertical level ("vector"/"gpsimd")
#   OUT_ENGINE: engine issuing the small result DMAs ("gpsimd"/"sync")
ROUNDS = 2
H_ENGINE = "vector"
V2_ENGINE = "vector"
OUT_ENGINE = "gpsimd"


def _build():
    import concourse.tile as tile
    from concourse import bacc, mybir

    f32 = mybir.dt.float32
    u32 = mybir.dt.uint32

    nc = bacc.Bacc("TRN2", target_bir_lowering=False, debug=False)
    x = nc.dram_tensor("x", [MAPS_PER_CORE, H, W], f32, kind="ExternalInput")
    vals = nc.dram_tensor(
        "vals", [MAPS_PER_CORE, 128, 16], f32, kind="ExternalOutput"
    )
    idxs = nc.dram_tensor(
        "idxs", [MAPS_PER_CORE, 128, 16], u32, kind="ExternalOutput"
    )

    with tile.TileContext(nc) as tc:
        with (
            tc.tile_pool(name="inp", bufs=3) as inp,
            tc.tile_pool(name="mid", bufs=2) as mid,
            tc.tile_pool(name="outp", bufs=3) as outp,
        ):
            for m in range(MAPS_PER_CORE):
                # ---- load map: partition p <- pixel rows 8p..8p+7 (32 KiB
                # contiguous per partition); 4 DMAs to spread across queues.
                t = inp.tile([128, 8192], f32)
                src = x[m].rearrange("(p k) j -> p k j", k=8)  # [128, 8, 1024]
                for g in range(4):
                    nc.sync.dma_start(
                        out=t[:, g * 2048 : (g + 1) * 2048],
                        in_=src[:, 2 * g : 2 * g + 2, :],
                    )

                # ---- vertical 4:1 within cells: rows (4r..4r+3) -> cell row
                # free-dim chunk k holds pixel row 8p+k. Pair (0,1),(2,3),...
                a = mid.tile([128, 4096], f32)
                for u in range(4):
                    nc.vector.tensor_max(
                        a[:, u * 1024 : (u + 1) * 1024],
                        t[:, (2 * u) * 1024 : (2 * u + 1) * 1024],
                        t[:, (2 * u + 1) * 1024 : (2 * u + 2) * 1024],
                    )
                # a chunks: [r01, r23, r45, r67]; combine to v = [cellrow 2p | 2p+1]
                v_eng = getattr(nc, V2_ENGINE)
                h_eng = getattr(nc, H_ENGINE)
                out_eng = getattr(nc, OUT_ENGINE)
                v = mid.tile([128, 2048], f32)
                a4 = a[:].rearrange("p (u j) -> p u j", j=1024)  # [128,4,1024]
                v_eng.tensor_max(v[:], a4[:, 0::2, :], a4[:, 1::2, :])

                # ---- horizontal 4:1: cols (4c..4c+3) -> cell col
                h1 = mid.tile([128, 1024], f32)
                v2 = v[:].rearrange("p (j two) -> p j two", two=2)  # [128,1024,2]
                h_eng.tensor_max(h1[:], v2[:, :, 0], v2[:, :, 1])
                cm = mid.tile([128, 512], f32)
                h2 = h1[:].rearrange("p (j two) -> p j two", two=2)
                h_eng.tensor_max(cm[:], h2[:, :, 0], h2[:, :, 1])

                # ---- top-8/16 cells per partition (max8 rounds);
                # separate tiles per round so each out-DMA has one writer.
                mv1 = outp.tile([128, 8], f32)
                mi1 = outp.tile([128, 8], u32)
                nc.vector.max(mv1[:], cm[:])
                nc.vector.max_index(mi1[:], mv1[:], cm[:])
                out_eng.dma_start(out=vals[m, :, 0:8], in_=mv1[:])
                out_eng.dma_start(out=idxs[m, :, 0:8], in_=mi1[:])
                if ROUNDS == 2:
                    mv2 = outp.tile([128, 8], f32)
                    mi2 = outp.tile([128, 8], u32)
                    cm2 = mid.tile([128, 512], f32)
                    nc.vector.match_replace(cm2[:], mv1[:], cm[:], NEG)
                    nc.vector.max(mv2[:], cm2[:])
                    nc.vector.max_index(mi2[:], mv2[:], cm2[:])
                    out_eng.dma_start(out=vals[m, :, 8:16], in_=mv2[:])
                    out_eng.dma_start(out=idxs[m, :, 8:16], in_=mi2[:])
    nc.compile()
    return nc


def _get_nc():
    global _NC
    if _NC is None:
        _NC = _build()
    return _NC


def run_device(xpad: np.ndarray, trace: bool = False):
    """Run the Bass kernel on 8 cores. xpad: [104, H, W] f32.

    Returns (vals [104,128,16] f32, idxs [104,128,16] uint32, results obj).
    """
    from concourse.bass_utils import run_bass_kernel_spmd

    nc = _get_nc()
    in_maps = [
        {"x": xpad[i * MAPS_PER_CORE : (i + 1) * MAPS_PER_CORE]}
        for i in range(NCORES)
    ]
    res = run_bass_kernel_spmd(nc, in_maps, list(range(NCORES)), trace=trace)
    vals = np.concatenate([r["vals"] for r in res.results], axis=0)
    idxs = np.concatenate([r["idxs"] for r in res.results], axis=0)
    return vals, idxs, res


def postprocess(x: np.ndarray, vals: np.ndarray, idxs: np.ndarray):
    """Exact NMS + stable top-100 from candidate cells.

    x: [NMAPS, H, W] f32 full input; vals/idxs: [NMAPS, 128, 16].
    """
    ncand = 128 * 16
    m_id = np.repeat(np.arange(NMAPS, dtype=np.int64), ncand)
    p = np.tile(np.repeat(np.arange(128, dtype=np.int64), 16), NMAPS)
    f = idxs[:NMAPS].reshape(-1).astype(np.int64)
    v_cand = vals[:NMAPS].reshape(-1)

    # Per-map trim to top-400 cells by value (candidates are cell maxima;
    # top-100 peaks lie within the top ~110 cells).
    keep = np.zeros(v_cand.size, dtype=bool)
    vc2 = v_cand.reshape(NMAPS, ncand)
    kth = np.partition(vc2, ncand - 400, axis=1)[:, ncand - 400]
    keep = (vc2 >= np.maximum(kth, THR)[:, None]).reshape(-1)
    m_id, p, f, v_cand = m_id[keep], p[keep], f[keep], v_cand[keep]

    # Decode cell coordinates: partition p holds cell rows 2p (f<256) and
    # 2p+1 (f>=256); cell col = f % 256.
    cr = 2 * p + (f >= 256)
    cc = f % 256

    # Pixels achieving the cell max (includes exact ties).
    d4 = np.arange(4, dtype=np.int64)
    py = 4 * cr[:, None, None] + d4[None, :, None]
    px = 4 * cc[:, None, None] + d4[None, None, :]
    patch = x[m_id[:, None, None], py, px]  # [N,4,4]
    pmax = patch.max(axis=(1, 2))
    sel = patch == pmax[:, None, None]
    ci, iy, ix = np.nonzero(sel)
    my = m_id[ci]
    yy = 4 * cr[ci] + iy
    xx = 4 * cc[ci] + ix
    vv = x[my, yy, xx]

    good = vv > THR
    my, yy, xx, vv = my[good], yy[good], xx[good], vv[good]

    # Exact 7x7 window max (border-clipped).
    wm = np.full(vv.shape, -np.inf, dtype=np.float32)
    for dy in range(-MIN_DISTANCE, MIN_DISTANCE + 1):
        y2 = yy + dy
        oky = (y2 >= 0) & (y2 < H)
        y2c = np.clip(y2, 0, H - 1)
        for dx in range(-MIN_DISTANCE, MIN_DISTANCE + 1):
            x2 = xx + dx
            ok = oky & (x2 >= 0) & (x2 < W)
            nb = x[my, y2c, np.clip(x2, 0, W - 1)]
            np.maximum(wm, np.where(ok, nb, -np.inf), out=wm)
    is_peak = vv == wm
    my, yy, xx, vv = my[is_peak], yy[is_peak], xx[is_peak], vv[is_peak]

    flat = yy * W + xx
    skeletons = np.zeros((NMAPS, K, 3), dtype=np.int32)
    scores = np.full((NMAPS, K), -np.inf, dtype=np.float32)
    order_all = np.argsort(my, kind="stable")
    my, flat, vv = my[order_all], flat[order_all], vv[order_all]
    bounds = np.searchsorted(my, np.arange(NMAPS + 1))
    for m in range(NMAPS):
        lo, hi = bounds[m], bounds[m + 1]
        fl, vm = flat[lo:hi], vv[lo:hi]
        if fl.size < K:
            raise RuntimeError(
                f"map {m}: only {fl.size} candidate peaks (< {K})"
            )
        o = np.lexsort((fl, -vm))[:K]
        fk, vk = fl[o], vm[o]
        seg = m % S
        skeletons[m, :, 0] = seg
        skeletons[m, :, 1] = (fk % W).astype(np.int32)
        skeletons[m, :, 2] = (fk // W).astype(np.int32)
        scores[m] = vk
    return (
        skeletons.reshape(B, S, K, 3),
        scores.reshape(B, S, K),
    )


def kernel(belive_map):
    x = np.ascontiguousarray(np.asarray(belive_map, dtype=np.float32)).reshape(
        NMAPS, H, W
    )
    xpad = np.concatenate(
        [x, np.zeros((NCORES * MAPS_PER_CORE - NMAPS, H, W), dtype=np.float32)],
        axis=0,
    )
    vals, idxs, _ = run_device(xpad, trace=False)
    return postprocess(x, vals, idxs)


# revision 12
# speedup vs baseline: 1.2244x; 1.0295x over previous
"""Trainium2 Bass kernel for BeliveMapsNMS (7x7 NMS + per-map top-100 peaks).

Strategy
--------
Input is belive_map [4, 25, 1024, 1024] f32 (400 MiB). The output is tiny
(top-100 local maxima per (b, s) map), so the kernel is a streaming
reduction bounded by HBM read bandwidth (memory regime).

Key algebraic fact: a pixel that is the max of its 7x7 window is always the
max of its aligned 4x4 cell (cell diameter 3 <= window radius 3). So a 16:1
cell-max reduction preserves every NMS peak. On each core:

  per map [1024,1024]:
    - load as one [128, 8192] SBUF tile (partition p holds pixel rows 8p..8p+7)
    - 7 tensor_tensor max ops reduce to 4x4 cell maxima cm [128, 512]
      (partition p holds cell rows 2p, 2p+1; 256 cell cols each)
    - 2 rounds of (max8 -> max_index -> match_replace) extract the top-16
      cells per partition (values + in-row indices)

Each (b,s) map contributes >= its top-16-cells-per-8-row-stripe; the global
top-100 peaks of a map are spread across 128 stripes (expected <1 per
stripe), so top-16 per stripe is a vast superset of the needed candidates.

Sharding: 100 maps flattened, padded to 104 = 8 cores x 13 maps. Pure data
parallel, no collectives.

Host post-pass (O(candidates), ~0.05% of input): for ~2048 candidate cells
per map, read the 4x4 patch from the (host-resident) input, keep pixels
achieving the cell max, check the exact 7x7 window, then stable top-100 by
(value desc, flat index asc) to match lax.top_k tie-breaking.
"""

import numpy as np

B, S, H, W = 4, 25, 1024, 1024
NMAPS = B * S            # 100
NCORES = 8
MAPS_PER_CORE = 13       # 8 * 13 = 104 (4 zero-padded maps)
K = 100
MIN_DISTANCE = 3
THR = np.float32(2.0 / (H * W))
NEG = -3.0e38

_NC = None

# Tuning knobs (variant-bench'd on HW):
#   ROUNDS: max8 extraction rounds (1 round = top-8/partition, 2 = top-16)
#   H_ENGINE: engine for the two horizontal reduce levels ("vector"/"gpsimd")
#   V2_ENGINE: engine for the 2nd vertical level ("vector"/"gpsimd")
#   OUT_ENGINE: engine issuing the small result DMAs ("gpsimd"/"sync")
ROUNDS = 1
H_ENGINE = "vector"
V2_ENGINE = "vector"
OUT_ENGINE = "gpsimd"
A_GPSIMD = 0  # how many of the 4 contiguous vertical max ops go to gpsimd
DMA_ACCUM = False  # fold 2 vertical max levels into max-accumulate DMA loads


def _build():
    import concourse.tile as tile
    from concourse import bacc, mybir

    f32 = mybir.dt.float32
    u32 = mybir.dt.uint32

    nc = bacc.Bacc("TRN2", target_bir_lowering=False, debug=False)
    x = nc.dram_tensor("x", [MAPS_PER_CORE, H, W], f32, kind="ExternalInput")
    vals = nc.dram_tensor(
        "vals", [MAPS_PER_CORE, 128, 16], f32, kind="ExternalOutput"
    )
    idxs = nc.dram_tensor(
        "idxs", [MAPS_PER_CORE, 128, 16], u32, kind="ExternalOutput"
    )

    with tile.TileContext(nc) as tc:
        with (
            tc.tile_pool(name="inp", bufs=3) as inp,
            tc.tile_pool(name="mid", bufs=2) as mid,
            tc.tile_pool(name="outp", bufs=3) as outp,
        ):
            for m in range(MAPS_PER_CORE):
                v_eng = getattr(nc, V2_ENGINE)
                h_eng = getattr(nc, H_ENGINE)
                out_eng = getattr(nc, OUT_ENGINE)
                v = mid.tile([128, 2048], f32)
                src = x[m].rearrange("(p k) j -> p (k j)", k=8)  # [128, 8192]
                if DMA_ACCUM:
                    # partition p rows 8p..8p+7 as 4 contiguous 2-row blocks;
                    # max-accumulate block pairs in the DMA engines:
                    # u[:, 0:2048]    = max(rows{0,1}, rows{2,3})
                    # u[:, 2048:4096] = max(rows{4,5}, rows{6,7})
                    u = inp.tile([128, 4096], f32)
                    for half in range(2):
                        dst = u[:, half * 2048 : (half + 1) * 2048]
                        base = half * 4096
                        nc.gpsimd.dma_start(
                            out=dst, in_=src[:, base : base + 2048]
                        )
                        nc.gpsimd.dma_start(
                            out=dst,
                            in_=src[:, base + 2048 : base + 4096],
                            accum_op=mybir.AluOpType.max,
                        )
                    # u chunks (1024 each): [max(r0,r2)|max(r1,r3)|max(r4,r6)|max(r5,r7)]
                    u4 = u[:].rearrange("p (u j) -> p u j", j=1024)
                    v_eng.tensor_max(v[:], u4[:, 0::2, :], u4[:, 1::2, :])
                else:
                    # ---- load map: partition p <- pixel rows 8p..8p+7 (32 KiB
                    # contiguous per partition); 4 DMAs to spread across queues.
                    t = inp.tile([128, 8192], f32)
                    for g in range(4):
                        nc.sync.dma_start(
                            out=t[:, g * 2048 : (g + 1) * 2048],
                            in_=src[:, g * 2048 : (g + 1) * 2048],
                        )

                    # ---- vertical 4:1 within cells: rows (4r..4r+3) -> cell
                    # row; free-dim chunk k holds pixel row 8p+k.
                    a = mid.tile([128, 4096], f32)
                    for uu in range(4):
                        a_eng = nc.gpsimd if uu < A_GPSIMD else nc.vector
                        a_eng.tensor_max(
                            a[:, uu * 1024 : (uu + 1) * 1024],
                            t[:, (2 * uu) * 1024 : (2 * uu + 1) * 1024],
                            t[:, (2 * uu + 1) * 1024 : (2 * uu + 2) * 1024],
                        )
                    # a chunks [r01,r23,r45,r67] -> v = [cellrow 2p | 2p+1]
                    a4 = a[:].rearrange("p (u j) -> p u j", j=1024)
                    v_eng.tensor_max(v[:], a4[:, 0::2, :], a4[:, 1::2, :])

                # ---- horizontal 4:1: cols (4c..4c+3) -> cell col
                h1 = mid.tile([128, 1024], f32)
                v2 = v[:].rearrange("p (j two) -> p j two", two=2)  # [128,1024,2]
                h_eng.tensor_max(h1[:], v2[:, :, 0], v2[:, :, 1])
                cm = mid.tile([128, 512], f32)
                h2 = h1[:].rearrange("p (j two) -> p j two", two=2)
                h_eng.tensor_max(cm[:], h2[:, :, 0], h2[:, :, 1])

                # ---- top-8/16 cells per partition (max8 rounds);
                # separate tiles per round so each out-DMA has one writer.
                mv1 = outp.tile([128, 8], f32)
                mi1 = outp.tile([128, 8], u32)
                nc.vector.max(mv1[:], cm[:])
                nc.vector.max_index(mi1[:], mv1[:], cm[:])
                out_eng.dma_start(out=vals[m, :, 0:8], in_=mv1[:])
                out_eng.dma_start(out=idxs[m, :, 0:8], in_=mi1[:])
                if ROUNDS == 2:
                    mv2 = outp.tile([128, 8], f32)
                    mi2 = outp.tile([128, 8], u32)
                    cm2 = mid.tile([128, 512], f32)
                    nc.vector.match_replace(cm2[:], mv1[:], cm[:], NEG)
                    nc.vector.max(mv2[:], cm2[:])
                    nc.vector.max_index(mi2[:], mv2[:], cm2[:])
                    out_eng.dma_start(out=vals[m, :, 8:16], in_=mv2[:])
                    out_eng.dma_start(out=idxs[m, :, 8:16], in_=mi2[:])
    nc.compile()
    return nc


def _get_nc():
    global _NC
    if _NC is None:
        _NC = _build()
    return _NC


def run_device(xpad: np.ndarray, trace: bool = False):
    """Run the Bass kernel on 8 cores. xpad: [104, H, W] f32.

    Returns (vals [104,128,16] f32, idxs [104,128,16] uint32, results obj).
    """
    from concourse.bass_utils import run_bass_kernel_spmd

    nc = _get_nc()
    in_maps = [
        {"x": xpad[i * MAPS_PER_CORE : (i + 1) * MAPS_PER_CORE]}
        for i in range(NCORES)
    ]
    res = run_bass_kernel_spmd(nc, in_maps, list(range(NCORES)), trace=trace)
    vals = np.concatenate([r["vals"] for r in res.results], axis=0)
    idxs = np.concatenate([r["idxs"] for r in res.results], axis=0)
    return vals, idxs, res


def postprocess(x: np.ndarray, vals: np.ndarray, idxs: np.ndarray):
    """Exact NMS + stable top-100 from candidate cells.

    x: [NMAPS, H, W] f32 full input; vals/idxs: [NMAPS, 128, 16].
    """
    ncand = 128 * 16
    m_id = np.repeat(np.arange(NMAPS, dtype=np.int64), ncand)
    p = np.tile(np.repeat(np.arange(128, dtype=np.int64), 16), NMAPS)
    f = idxs[:NMAPS].reshape(-1).astype(np.int64)
    v_cand = vals[:NMAPS].reshape(-1)

    # Per-map trim to top-400 cells by value (candidates are cell maxima;
    # top-100 peaks lie within the top ~110 cells).
    keep = np.zeros(v_cand.size, dtype=bool)
    vc2 = v_cand.reshape(NMAPS, ncand)
    kth = np.partition(vc2, ncand - 400, axis=1)[:, ncand - 400]
    keep = (vc2 >= np.maximum(kth, THR)[:, None]).reshape(-1)
    m_id, p, f, v_cand = m_id[keep], p[keep], f[keep], v_cand[keep]

    # Decode cell coordinates: partition p holds cell rows 2p (f<256) and
    # 2p+1 (f>=256); cell col = f % 256.
    cr = 2 * p + (f >= 256)
    cc = f % 256

    # Pixels achieving the cell max (includes exact ties).
    d4 = np.arange(4, dtype=np.int64)
    py = 4 * cr[:, None, None] + d4[None, :, None]
    px = 4 * cc[:, None, None] + d4[None, None, :]
    patch = x[m_id[:, None, None], py, px]  # [N,4,4]
    pmax = patch.max(axis=(1, 2))
    sel = patch == pmax[:, None, None]
    ci, iy, ix = np.nonzero(sel)
    my = m_id[ci]
    yy = 4 * cr[ci] + iy
    xx = 4 * cc[ci] + ix
    vv = x[my, yy, xx]

    good = vv > THR
    my, yy, xx, vv = my[good], yy[good], xx[good], vv[good]

    # Exact 7x7 window max (border-clipped).
    wm = np.full(vv.shape, -np.inf, dtype=np.float32)
    for dy in range(-MIN_DISTANCE, MIN_DISTANCE + 1):
        y2 = yy + dy
        oky = (y2 >= 0) & (y2 < H)
        y2c = np.clip(y2, 0, H - 1)
        for dx in range(-MIN_DISTANCE, MIN_DISTANCE + 1):
            x2 = xx + dx
            ok = oky & (x2 >= 0) & (x2 < W)
            nb = x[my, y2c, np.clip(x2, 0, W - 1)]
            np.maximum(wm, np.where(ok, nb, -np.inf), out=wm)
    is_peak = vv == wm
    my, yy, xx, vv = my[is_peak], yy[is_peak], xx[is_peak], vv[is_peak]

    flat = yy * W + xx
    skeletons = np.zeros((NMAPS, K, 3), dtype=np.int32)
    scores = np.full((NMAPS, K), -np.inf, dtype=np.float32)
    order_all = np.argsort(my, kind="stable")
    my, flat, vv = my[order_all], flat[order_all], vv[order_all]
    bounds = np.searchsorted(my, np.arange(NMAPS + 1))
    for m in range(NMAPS):
        lo, hi = bounds[m], bounds[m + 1]
        fl, vm = flat[lo:hi], vv[lo:hi]
        if fl.size < K:
            raise RuntimeError(
                f"map {m}: only {fl.size} candidate peaks (< {K})"
            )
        o = np.lexsort((fl, -vm))[:K]
        fk, vk = fl[o], vm[o]
        seg = m % S
        skeletons[m, :, 0] = seg
        skeletons[m, :, 1] = (fk % W).astype(np.int32)
        skeletons[m, :, 2] = (fk // W).astype(np.int32)
        scores[m] = vk
    return (
        skeletons.reshape(B, S, K, 3),
        scores.reshape(B, S, K),
    )


def kernel(belive_map):
    x = np.ascontiguousarray(np.asarray(belive_map, dtype=np.float32)).reshape(
        NMAPS, H, W
    )
    xpad = np.concatenate(
        [x, np.zeros((NCORES * MAPS_PER_CORE - NMAPS, H, W), dtype=np.float32)],
        axis=0,
    )
    vals, idxs, _ = run_device(xpad, trace=False)
    return postprocess(x, vals, idxs)
